# revision 30
# baseline (speedup 1.0000x reference)
"""Trainium2 Bass kernel for nn_BinaryLinear (8-core SPMD) — v5.

z = x @ binarize(W).T + binarize(b); out = relu((z - mean)/(std + eps))

v5 over v4 (prep-phase restructure; main loop kept):
  - All big HBM loads share the sync queue so ordering is a true FIFO:
    w pass-1 (8MB) -> w pass-2 (8MB) -> x (16MB). v4's cross-queue dep
    only ordered instruction issue, so x competed with w for HBM and the
    threshold (which gates everything) slipped from ~30us to ~100us.
  - Threshold chain on the Pool engine: per-chunk reduce_sum ->
    partition_all_reduce (no DRAM round-trips) -> tiny AllReduce.
    Bias threshold likewise.
  - w binarize (is_gt) on Pool, freeing DVE for the x lo-evictions.
  - AllGather stages issue at ~70us (v4: ~171us); pass A's jj=0 runs
    full-k from the resident own-block transpose to cover AG1 flight.
  - PE queue emission interleaves x-transposes, w-transposes and the
    jj=0 matmul groups in expected execution order (in-order queue).
  - Pass B runs m-pairs (0,1),(2,3),(4,5) then singles 6, 7 so the tail
    after the last matmul only normalizes/stores one m-row, with the
    final stores split across queues.
"""
import numpy as np

import concourse.bass as bass
import concourse.mybir as mybir
import concourse.tile as tile
from concourse import bacc
from concourse import bass_isa
from concourse.bass_utils import run_bass_kernel_spmd
from concourse.masks import make_identity

N_CORES = 8
T_FULL = 8192
D_IN = 4096
D_OUT = 4096
T_SHARD = T_FULL // N_CORES    # 1024
O_SHARD = D_OUT // N_CORES     # 512
P = 128
NK = D_IN // P                 # 32 k-tiles
NKH = NK // 2                  # 16 per AG stage
NM = T_SHARD // P              # 8 token tiles
NJ = D_OUT // O_SHARD          # 8 o-blocks
QW = D_IN // 4                 # 1024
EPS = 1e-5
F32 = mybir.dt.float32
BF16 = mybir.dt.bfloat16
FP8 = mybir.dt.float8e4
DR = mybir.MatmulPerfMode.DoubleRow

_cache: dict = {}
last_exec_time_ns = None


def _pair0(ap2d):
    return bass.AP(tensor=ap2d.tensor, offset=ap2d.offset,
                   ap=[ap2d.ap[0], [0, 2], ap2d.ap[-1]])


def _bcast(ap, n_part, width=1):
    return bass.AP(tensor=ap.tensor, offset=ap.offset,
                   ap=[[0, n_part], [1, width]])


def _build():
    nc = bacc.Bacc("TRN2", target_bir_lowering=False, debug=False,
                   num_devices=N_CORES)
    x_in = nc.dram_tensor("x", [T_SHARD, D_IN], F32, kind="ExternalInput")
    w_in = nc.dram_tensor("w", [O_SHARD, D_IN], F32, kind="ExternalInput")
    b_in = nc.dram_tensor("b", [D_OUT], F32, kind="ExternalInput")
    out_ext = nc.dram_tensor("out", [T_SHARD, D_OUT], F32, kind="ExternalOutput")

    with tile.TileContext(nc) as tc:
        with (
            tc.tile_pool(name="xT_pool", bufs=1) as xT_pool,
            tc.tile_pool(name="z_pool", bufs=1) as z_pool,
            tc.tile_pool(name="wtr_pool", bufs=1) as wtr_pool,
            tc.tile_pool(name="wg_pool", bufs=4) as wg_pool,
            tc.tile_pool(name="wstage", bufs=2) as wstage,
            tc.tile_pool(name="wqb_pool", bufs=2) as wqb_pool,
            tc.tile_pool(name="xbf_pool", bufs=2) as xbf_pool,
            tc.tile_pool(name="bias_pool", bufs=1) as bias_pool,
            tc.tile_pool(name="small", bufs=1) as small,
            tc.tile_pool(name="psum", bufs=4, space="PSUM") as psum,
            tc.tile_pool(name="ptr_w", bufs=2, space="PSUM") as ptr_w,
            tc.tile_pool(name="ptr_x", bufs=2, space="PSUM") as ptr_x,
            tc.tile_pool(name="dram", bufs=1, space="DRAM") as dram,
        ):
            # xT pairs: ksub 2k = hi(e4m3), 2k+1 = lo; cols m*128..(m+1)*128
            xT = xT_pool.tile([P, 2 * NK, T_SHARD], FP8)               # 8 MB
            z_sb = [z_pool.tile([P, D_OUT], BF16, name=f"z{m}")        # 8 MB
                    for m in range(NM)]
            wtr = wtr_pool.tile([P, NK, O_SHARD], FP8)                 # 2 MB

            identity = small.tile([P, P], BF16)
            make_identity(nc, identity)

            # ---- A: w single load: 16 f32 chunks split across the sync and
            #      scalar queues (2x HBM feed). Per chunk: DVE partial-sum
            #      reduce AND an ACT cast to bf16 cached in z_sb[c4] (the z
            #      rows are dead until the first eviction at ~50us, and a w
            #      c4-chunk is exactly a z-row). No w reload needed. ----
            colsums = small.tile([P, 16], F32)
            last_wred = None
            for ch in range(16):
                c4, hh = ch // 4, ch % 4
                wf = wstage.tile([P, QW], F32, name=f"wf{ch}", tag="ws")
                nc.sync.dma_start(
                    out=wf[:],
                    in_=w_in[c4 * P:(c4 + 1) * P,
                             hh * QW:(hh + 1) * QW])
                last_wred = nc.vector.reduce_sum(
                    colsums[:, ch:ch + 1], wf[:], axis=mybir.AxisListType.X)
                nc.scalar.copy(
                    out=z_sb[c4][:, hh * QW:(hh + 1) * QW], in_=wf[:])

            # ---- B: threshold: rowsum -> DRAM round-trip (partition fold)
            #      -> AllGather of the 8 shard sums (15us vs AllReduce's
            #      28us) -> local sum. Small DMAs ride the sync queue. ----
            rowsum = small.tile([P, 1], F32)
            nc.vector.reduce_sum(rowsum[:], colsums[:],
                                 axis=mybir.AxisListType.X)
            rs_dram = dram.tile([P], F32)
            nc.sync.dma_start(
                out=rs_dram[:].rearrange("(p o) -> p o", p=P), in_=rowsum[:])
            rs_row = small.tile([1, P], F32)
            nc.sync.dma_start(
                out=rs_row[:], in_=rs_dram[:].rearrange("(o p) -> o p", o=1))
            ar_sb = small.tile([1, 8], F32)
            nc.vector.memset(ar_sb[:], 0.0)
            nc.vector.reduce_sum(ar_sb[:, 0:1], rs_row[:],
                                 axis=mybir.AxisListType.X)
            ar_in = dram.tile([8], F32)
            ar_out = dram.tile([N_CORES, 8], F32, addr_space="Shared")
            nc.sync.dma_start(out=ar_in[:].rearrange("(o d) -> o d", o=1),
                              in_=ar_sb[:])

            # ---- E: bias binarize ([128, 32] layout). All DMAs on sync so
            #      the scalar queue stays compute-only (the w casts). ----
            bsb = small.tile([P, D_OUT // P], F32)
            nc.sync.dma_start(
                out=bsb[:], in_=b_in[:].rearrange("(p j) -> p j", p=P))
            bpart = small.tile([P, 1], F32)
            nc.vector.reduce_sum(bpart[:], bsb[:], axis=mybir.AxisListType.X)
            bp_dram = dram.tile([P], F32)
            nc.sync.dma_start(
                out=bp_dram[:].rearrange("(p o) -> p o", p=P), in_=bpart[:])
            bp_row = small.tile([1, P], F32)
            nc.sync.dma_start(
                out=bp_row[:], in_=bp_dram[:].rearrange("(o p) -> o p", o=1))
            b_mean1 = small.tile([1, 1], F32)
            nc.vector.reduce_sum(b_mean1[:], bp_row[:],
                                 axis=mybir.AxisListType.X)
            nc.vector.tensor_scalar_mul(b_mean1[:], b_mean1[:], 1.0 / D_OUT)
            bm_dram = dram.tile([1], F32)
            nc.sync.dma_start(out=bm_dram[:].rearrange("(o d) -> o d", o=1),
                              in_=b_mean1[:])
            b_mean = small.tile([P, 1], F32)
            nc.sync.dma_start(out=b_mean[:], in_=_bcast(bm_dram.opt(), P))
            b_q128 = small.tile([P, D_OUT // P], FP8)
            nc.vector.tensor_scalar(
                out=b_q128[:], in0=bsb[:], scalar1=b_mean[:], scalar2=None,
                op0=mybir.AluOpType.is_gt,
            )
            bq_dram = dram.tile([D_OUT], FP8)
            nc.sync.dma_start(
                out=bq_dram[:].rearrange("(p j) -> p j", p=P), in_=b_q128[:])
            bias_bcast = bias_pool.tile([P, D_OUT], FP8)                # 0.5 MB
            nc.sync.dma_start(
                out=bias_bcast[:],
                in_=bass.AP(tensor=bq_dram.opt().tensor, offset=0,
                            ap=[[0, P], [1, D_OUT]]))

            # ---- C: x loads: casting DMAs must use the gpsimd queue.
            #      First one waits on the last w reduce (which completes
            #      only after the last w transfer), so w wins HBM for
            #      real; the enqueues finish fast so the thr-AllGather
            #      emitted next dispatches promptly. ----
            xbf = {}
            x_first = None
            for mx in range(NM):
                for xh in range(2):
                    t = xbf_pool.tile([P, 2 * QW], BF16,
                                      name=f"xbf{mx}_{xh}", tag="xbf")
                    xd = nc.gpsimd.dma_start(
                        out=t[:],
                        in_=x_in[mx * P:(mx + 1) * P,
                                 xh * 2 * QW:(xh + 1) * 2 * QW])
                    if x_first is None:
                        x_first = xd
                        tile.add_dep_helper(
                            xd.ins, last_wred.ins, sync=True,
                            reason="w transfers win HBM first")
                    xbf[(mx, xh)] = t

            nc.gpsimd.collective_compute(
                "AllGather", mybir.AluOpType.bypass,
                replica_groups=[list(range(N_CORES))],
                ins=[ar_in.opt()], outs=[ar_out.opt()],
            )
            thr8 = small.tile([P, N_CORES], F32)
            nc.sync.dma_start(
                out=thr8[:],
                in_=bass.AP(tensor=ar_out.opt().tensor, offset=0,
                            ap=[[0, P], [8, N_CORES], [1, 1]]))
            thr_sb = small.tile([P, 1], F32)
            nc.vector.reduce_sum(thr_sb[:], thr8[:],
                                 axis=mybir.AxisListType.X)
            nc.vector.tensor_scalar_mul(thr_sb[:], thr_sb[:],
                                        1.0 / (D_OUT * D_IN))

            # ---- helpers: x transpose+evict for one m; w binarize+transpose
            #      for one (half, c4). Emission order = engine order. ----
            def emit_xT(mx):
                for xh in range(2):
                    src = xbf[(mx, xh)]
                    for q in range(2):
                        pt = ptr_x.tile([P, 8 * P], BF16,
                                        name=f"xpt{mx}_{xh}_{q}", tag="ptrx")
                        for i in range(8):
                            nc.tensor.transpose(
                                pt[:, i * P:(i + 1) * P],
                                src[:, (q * 8 + i) * P:(q * 8 + i + 1) * P],
                                identity[:])
                        k0 = xh * NKH + q * 8
                        xap = xT[:]
                        hi = bass.AP(tensor=xap.tensor,
                                     offset=(2 * k0) * T_SHARD + mx * P,
                                     ap=[xap.ap[0], [2 * T_SHARD, 8], [1, P]])
                        lo = bass.AP(tensor=xap.tensor,
                                     offset=(2 * k0 + 1) * T_SHARD + mx * P,
                                     ap=[xap.ap[0], [2 * T_SHARD, 8], [1, P]])
                        nc.scalar.copy(out=hi, in_=pt[:])
                        nc.vector.tensor_tensor(
                            out=lo, in0=pt[:], in1=hi,
                            op=mybir.AluOpType.subtract)

            def emit_w_chunk(h, c4, q2):
                """Binarize one bf16 w chunk from its z_sb cache, transpose
                its 8 k-tiles into one psum bank, evict to wtr (fp8)."""
                wq = wqb_pool.tile([P, QW], BF16,
                                   name=f"wq{c4}_{h}_{q2}", tag="wqb")
                nc.vector.tensor_scalar(
                    out=wq[:],
                    in0=z_sb[c4][:, (2 * h + q2) * QW:(2 * h + q2 + 1) * QW],
                    scalar1=thr_sb[:], scalar2=None,
                    op0=mybir.AluOpType.is_gt,
                )
                pt = ptr_w.tile([P, 8 * P], BF16,
                                name=f"wpt{h}_{c4}_{q2}", tag="ptrw")
                for i in range(8):
                    nc.tensor.transpose(
                        pt[:, i * P:(i + 1) * P],
                        wq[:, i * P:(i + 1) * P],
                        identity[:])
                k0 = h * NKH + q2 * 8
                nc.scalar.copy(
                    out=wtr[:, k0:k0 + 8, c4 * P:(c4 + 1) * P],
                    in_=pt[:])

            # PE queue order: x m0, m1 first (cheap, early data), then the
            # w transposes (binarize gates them on thr), then alternate
            # x-transposes with jj=0 matmul groups.
            emit_xT(0)
            emit_xT(1)
            for h in range(2):
                for c4 in range(4):
                    for q2 in range(2):
                        emit_w_chunk(h, c4, q2)

            # ---- F: AllGather of the transposed binarized shard ----
            ag_in = [dram.tile([P * NKH * O_SHARD], FP8, name=f"agi{h}")
                     for h in range(2)]
            ag_out = [dram.tile([N_CORES, P * NKH * O_SHARD], FP8,
                                name=f"ago{h}", addr_space="Shared")
                      for h in range(2)]
            for h in range(2):
                nc.sync.dma_start(
                    out=ag_in[h][:].rearrange("(p k o) -> p k o", p=P, k=NKH),
                    in_=wtr[:, h * NKH:(h + 1) * NKH, :])
                nc.gpsimd.collective_compute(
                    "AllGather", mybir.AluOpType.bypass,
                    replica_groups=[list(range(N_CORES))],
                    ins=[ag_in[h].opt()], outs=[ag_out[h].opt()],
                )

            # ---- G: main loop ----
            stats = [small.tile([P, NJ, 6], F32, name=f"stats{m}")
                     for m in range(NM)]
            pid_s = nc.sync.partition_id()
            pid_v = nc.vector.partition_id()

            def wg_load(h, jj, tag_name):
                """Load o-block (pid+jj)&7 of AG stage h into SBUF."""
                wg = wg_pool.tile([P, NKH, O_SHARD], FP8, name=tag_name,
                                  tag="wg")
                r = (pid_s + jj) & 7
                src = bass.AP(
                    tensor=ag_out[h].opt().tensor,
                    offset=r * (P * NKH * O_SHARD),
                    ap=[[NKH * O_SHARD, P], [O_SHARD, NKH], [1, O_SHARD]])
                nc.sync.dma_start(out=wg[:], in_=src)
                return wg

            def mm_group(ps, wg_ap_fn, m, ks, start, stop):
                for i, k in enumerate(ks):
                    nc.tensor.matmul(
                        ps[:],
                        xT[:, 2 * k:2 * k + 2, m * P:(m + 1) * P],
                        _pair0(wg_ap_fn(k)),
                        start=(start and i == 0),
                        stop=(stop and i == len(ks) - 1),
                        perf_mode=DR)

            def mm_group2(ps_a, fn_a, ps_b, fn_b, m, ks):
                """Two o-blocks sharing each stationary xT slice: the PE
                skips reloading an unchanged stationary, so the second
                matmul of each k runs load-free (~512 vs 640 cycles)."""
                for i, k in enumerate(ks):
                    st = xT[:, 2 * k:2 * k + 2, m * P:(m + 1) * P]
                    nc.tensor.matmul(ps_a[:], st, _pair0(fn_a(k)),
                                     start=(i == 0), stop=(i == len(ks) - 1),
                                     perf_mode=DR)
                    nc.tensor.matmul(ps_b[:], st, _pair0(fn_b(k)),
                                     start=(i == 0), stop=(i == len(ks) - 1),
                                     perf_mode=DR)

            def z_evict(m, jj, ps, with_stats, first):
                rv = ((pid_v + jj) & 7) * O_SHARD
                zrow = z_sb[m]
                nc.vector.tensor_tensor(
                    out=zrow[:, bass.ds(rv, O_SHARD)],
                    in0=ps[:],
                    in1=(bias_bcast[:, bass.ds(rv, O_SHARD)] if first
                         else zrow[:, bass.ds(rv, O_SHARD)]),
                    op=mybir.AluOpType.add)
                if with_stats:
                    nc.vector.bn_stats(
                        out=stats[m][:, jj, :],
                        in_=zrow[:, bass.ds(rv, O_SHARD)])

            # pass-A: jj=0 full-k solo from resident wtr (covers AG1
            # flight), then jj pairs (1,2),(3,4),(5,6) and jj=7 solo on
            # k-half 0. x transposes m2..m7 interleave into the PE queue.
            next_xT = 2
            wtr_fn = lambda k: wtr[:, k, :]
            for m in range(NM):
                ps = psum.tile([P, O_SHARD], F32, name=f"psA0_{m}", tag="ps")
                mm_group(ps, wtr_fn, m, list(range(NK)), True, True)
                if next_xT < NM:
                    emit_xT(next_xT)
                    next_xT += 1
                z_evict(m, 0, ps, True, True)

            ksA = list(range(NKH))
            for ja, jb in [(1, 2), (3, 4), (5, 6)]:
                wga = wg_load(0, ja, f"wgA{ja}")
                wgb = wg_load(0, jb, f"wgA{jb}")
                fn_a = (lambda w: lambda k: w[:, k, :])(wga)
                fn_b = (lambda w: lambda k: w[:, k, :])(wgb)
                for m in range(NM):
                    ps_a = psum.tile([P, O_SHARD], F32,
                                     name=f"psA{ja}_{m}", tag="ps")
                    ps_b = psum.tile([P, O_SHARD], F32,
                                     name=f"psA{jb}_{m}", tag="ps")
                    mm_group2(ps_a, fn_a, ps_b, fn_b, m, ksA)
                    z_evict(m, ja, ps_a, False, True)
                    z_evict(m, jb, ps_b, False, True)
            wg7 = wg_load(0, 7, "wgA7")
            fn7 = lambda k: wg7[:, k, :]
            for m in range(NM):
                ps = psum.tile([P, O_SHARD], F32, name=f"psA7_{m}", tag="ps")
                mm_group(ps, fn7, m, ksA, True, True)
                z_evict(m, 7, ps, False, True)

            # pass-B: k-half 1 for jj>=1. m-groups (0,1),(2,3),(4,5),(6),(7)
            # so each group's normalize + stores overlap the next group's
            # matmuls and the final tail is a single m-row.
            groups = [(0, 1), (2, 3), (4, 5), (6,), (7,)]
            ksB = list(range(NKH, NK))
            store_qs = [nc.scalar, nc.gpsimd]
            store_i = 0
            for gi, ms in enumerate(groups):
                for ja, jb in [(1, 2), (3, 4), (5, 6)]:
                    wga = wg_load(1, ja, f"wgB{gi}_{ja}")
                    wgb = wg_load(1, jb, f"wgB{gi}_{jb}")
                    fn_a = (lambda w: lambda k: w[:, k - NKH, :])(wga)
                    fn_b = (lambda w: lambda k: w[:, k - NKH, :])(wgb)
                    for m in ms:
                        ps_a = psum.tile([P, O_SHARD], F32,
                                         name=f"psB{gi}_{ja}_{m}", tag="ps")
                        ps_b = psum.tile([P, O_SHARD], F32,
                                         name=f"psB{gi}_{jb}_{m}", tag="ps")
                        mm_group2(ps_a, fn_a, ps_b, fn_b, m, ksB)
                        z_evict(m, ja, ps_a, True, False)
                        z_evict(m, jb, ps_b, True, False)
                wg7b = wg_load(1, 7, f"wgB{gi}_7")
                fn7b = lambda k: wg7b[:, k - NKH, :]
                for m in ms:
                    ps = psum.tile([P, O_SHARD], F32,
                                   name=f"psB{gi}_7_{m}", tag="ps")
                    mm_group(ps, fn7b, m, ksB, True, True)
                    z_evict(m, 7, ps, True, False)
                for m in ms:
                    zrow = z_sb[m]
                    mv = small.tile([P, 2], F32, name=f"mv{m}")
                    nc.vector.bn_aggr(out=mv[:], in_=stats[m][:])
                    std = small.tile([P, 1], F32, name=f"std{m}")
                    nc.scalar.sqrt(std[:], mv[:, 1:2])
                    nc.vector.tensor_scalar_add(std[:], std[:], EPS)
                    rstd = small.tile([P, 1], F32, name=f"rstd{m}")
                    nc.vector.reciprocal(rstd[:], std[:])
                    shift = small.tile([P, 1], F32, name=f"shift{m}")
                    nc.vector.tensor_mul(shift[:], mv[:, 0:1], rstd[:])
                    nc.vector.tensor_scalar_mul(shift[:], shift[:], -1.0)
                    for qh in range(4):
                        ot = wstage.tile([P, QW], F32,
                                         name=f"ot{m}_{qh}", tag="ws")
                        nc.scalar.activation(
                            out=ot[:],
                            in_=zrow[:, qh * QW:(qh + 1) * QW],
                            func=mybir.ActivationFunctionType.Relu,
                            bias=shift[:], scale=rstd[:],
                        )
                        store_qs[store_i % 2].dma_start(
                            out=out_ext[m * P:(m + 1) * P,
                                        qh * QW:(qh + 1) * QW],
                            in_=ot[:])
                        store_i += 1

    nc.finalize()
    return nc


def kernel(x: np.ndarray, weight: np.ndarray, b: np.ndarray) -> np.ndarray:
    global last_exec_time_ns
    import os
    x = np.ascontiguousarray(x, dtype=np.float32)
    weight = np.ascontiguousarray(weight, dtype=np.float32)
    b = np.ascontiguousarray(b, dtype=np.float32)
    assert x.shape == (T_FULL, D_IN) and weight.shape == (D_OUT, D_IN)

    if "nc" not in _cache:
        _cache["nc"] = _build()
    nc = _cache["nc"]

    in_maps = [
        {
            "x": x[c * T_SHARD:(c + 1) * T_SHARD],
            "w": weight[c * O_SHARD:(c + 1) * O_SHARD],
            "b": b,
        }
        for c in range(N_CORES)
    ]
    trace = os.environ.get("BASS_KERNEL_TRACE", "") == "1"
    res = run_bass_kernel_spmd(nc, in_maps, list(range(N_CORES)), trace=trace)
    last_exec_time_ns = res.exec_time_ns
    return np.concatenate([res.results[c]["out"] for c in range(N_CORES)],
                          axis=0)
